# revision 9
# baseline (speedup 1.0000x reference)
"""Fused OT-DTW l2 cost-matrix kernel for Trainium2 (8 NeuronCores, SPMD).

mat_cost[i,j] = sum_{t,p,d} pi[cl(i)][t,p] * (X[i,t,d] - Y[j,p,d])^2
             = C1[i] + C2[cl(i), j] - 2 * C3[i,j]

with C3[i,j] = sum_{p,d} XP[i,p,d] * Y[j,p,d],  XP[i] = X[i].T @ pi[cl(i)].
The device computes the heavy parts (XP: ~69 GFLOP, C3: ~137 GFLOP) in
fp8e4m3 (pi is 0/1 so fp8 is exact for it; X/Y quantization error washes
out over the 65536-term contraction). The tiny rank-1 corrections C1/C2
(<0.2% of FLOPs) are applied on the host in fp32.

Sharding (4 row-groups x 2 p-halves): core k = 2g + h takes 256 rows of X
and contraction half p in [256h, 256h+256). Each core emits the partial
C3 over its p-half; the host adds the two partials per group. This halves
the Y stream per core (33.5MB, under the stage-B PE time) and splits
stage A across cores with zero duplication, putting per-core PE work at
the global fp8 roofline (~164us).

Everything runs fp8 DoubleRow (contraction 256/instr, 2 fp8/lane). The
dual-fp8 ISA forbids register-offset APs on the moving operand, so class
selection is baked statically: rows are grouped into 256 quadruples of 4
same-class rows (one row per group per slot -> all cores share one
slot->class schedule; programs are cached per schedule). Remainder rows
(<=24) land in dummy slots whose C3 rows the host recomputes exactly.

Stage A makes pi the stationary operand (static class offsets) and
streams X 4 rows at a time: out[p_chunk,(d,i)] = pi_chunk.T @ X_batch,
4 matmuls of free-dim 1024 per quad (LDWEIGHTS fully hidden), then
corner-turn casts PSUM->SBUF xp[q_p, d, r_p, i] fp8 split DVE/ACT.
Stage B contracts with p on partitions: for each d, lhsT = xp[:, d, :,
ic], rhs = ytp tile [q_p, r_p, 4d, 1024j] streamed through a 6-buffer
ring; 512 DR matmuls of free-dim 1024 accumulate into 4 PSUM banks.
A scratch-matmul burst at t=0 warms the PE clock-gate.
"""

import os
import sys
import types

import numpy as np
import ml_dtypes

NX, NY, T, TP, D, C = 1024, 1024, 512, 512, 128, 8
N_CORES = 8
GX = 4                      # row groups
R = NX // GX                # 256 rows (slots) per core
PL = TP // 2                # 256 local p (contraction half)
TC = T // 128               # 4 t-chunks
DG = 4                      # d-slices per Y DMA tile
XG = 8                      # slots per xs DMA tile
BF16 = ml_dtypes.bfloat16
F8 = ml_dtypes.float8_e4m3fn


def _ensure_axon_hooks():
    """concourse.bass_utils imports antenv.axon_hooks when tracing under
    axon; some images lack that submodule. Provide it, and register the
    NTFF profile hook if the boot path didn't."""
    try:
        import antenv
    except ImportError:
        return
    try:
        from antenv import axon_hooks  # noqa: F401
    except ImportError:
        mod = types.ModuleType("antenv.axon_hooks")
        mod._hook = None

        def _set(h):
            mod._hook = h

        def _get():
            return mod._hook

        mod.set_axon_ntff_profile_hook = _set
        mod.get_axon_ntff_profile_hook = _get
        sys.modules["antenv.axon_hooks"] = mod
        antenv.axon_hooks = mod
    from antenv.axon_hooks import (
        get_axon_ntff_profile_hook,
        set_axon_ntff_profile_hook,
    )

    if get_axon_ntff_profile_hook() is None:
        try:
            from trn_agent_boot.trn_boot import _ntff_profile_via_ctypes

            hook = _ntff_profile_via_ctypes("/opt/axon/libaxon_pjrt.so")
            if hook is not None:
                set_axon_ntff_profile_hook(hook)
        except Exception:
            pass


_ensure_axon_hooks()

import concourse.bass as bass  # noqa: E402  (bass.ds unused but kept for parity)
import concourse.tile as tile  # noqa: E402
from concourse import bacc, mybir  # noqa: E402
from concourse.bass_utils import run_bass_kernel_spmd  # noqa: E402

_PROGRAM_CACHE = {}
LAST_RUN = None  # BassKernelResults of the most recent kernel() call


def _schedule(classe):
    """Group rows into 256 quadruples (one row per group per slot).

    Returns (slot_cls, perm, dummy_rows): slot_cls[s] = baked class of
    slot s (remainder slots labeled 0), perm[s, g] = original row id at
    slot s of group g, dummy_rows = rows whose device result is replaced
    by an exact host recompute (mixed-class remainder quadruples).
    """
    slot_cls, quads, leftovers = [], [], []
    for c in range(C):
        rows = np.flatnonzero(classe == c)
        n4 = len(rows) // 4 * 4
        for k in range(0, n4, 4):
            quads.append(rows[k:k + 4])
            slot_cls.append(c)
        leftovers.extend(rows[n4:])
    leftovers = np.asarray(leftovers, dtype=np.int64)
    assert len(leftovers) % 4 == 0
    for k in range(0, len(leftovers), 4):
        quads.append(leftovers[k:k + 4])
        slot_cls.append(0)
    perm = np.stack(quads)                      # [256, 4]
    assert perm.shape == (R, GX)
    return tuple(slot_cls), perm, leftovers


def _batches(slot_cls):
    """Static-class batch pieces on the 4-slot grid: (s0, w, cls)."""
    out = []
    for b in range(R // 4):
        s = 4 * b
        while s < 4 * b + 4:
            c = slot_cls[s]
            w = 1
            while s + w < 4 * b + 4 and slot_cls[s + w] == c:
                w += 1
            out.append((s, w, int(c)))
            s += w
    return out


def _build_program(slot_cls):
    if slot_cls in _PROGRAM_CACHE:
        return _PROGRAM_CACHE[slot_cls]
    f8 = mybir.dt.float8e4
    f32 = mybir.dt.float32
    DR = mybir.MatmulPerfMode.DoubleRow
    nc = bacc.Bacc("TRN2", target_bir_lowering=False, debug=False,
                   num_devices=N_CORES)
    xs = nc.dram_tensor("xs", [R, 128, 2, 2, D], f8, kind="ExternalInput").ap()
    pi_d = nc.dram_tensor("pi_d", [128, 2, 2, C * PL], f8,
                          kind="ExternalInput").ap()
    yt = nc.dram_tensor("yt", [128, 2, D, NY], f8, kind="ExternalInput").ap()
    c3 = nc.dram_tensor("c3", [R, NY], f32, kind="ExternalOutput").ap()

    with tile.TileContext(nc) as tc:
        with (
            tc.tile_pool(name="xpp", bufs=1) as xp_pool,
            tc.tile_pool(name="xin", bufs=8) as xin_pool,
            tc.tile_pool(name="pisb", bufs=1) as pi_pool,
            tc.tile_pool(name="yin", bufs=6) as y_pool,
            tc.tile_pool(name="outsb", bufs=1) as out_pool,
        ):
            # Resident transposed XP: xp[q_p, d, r_p, i] fp8 (64KB/part).
            xp = xp_pool.tile([128, D, 2, R], f8)

            # PE warmup: scratch matmuls at t=0 so the HAM clock-gate hits
            # 8/8 before the real matmuls start (values never read).
            with (
                tc.tile_pool(name="warm", bufs=1) as warm_pool,
                tc.tile_pool(name="warmps", bufs=1, space="PSUM") as warmps_pool,
            ):
                wsrc = warm_pool.tile([128, 512], f8)
                wacc = warmps_pool.tile([128, 512], f32)
                nc.gpsimd.memset(wsrc[:], 0.0)
                for w in range(14):
                    nc.tensor.matmul(wacc[:], wsrc[:, 0:128], wsrc[:],
                                     start=True, stop=True)

            # ---- Stage A: xp[:, :, pc, s] = pi_cl(s)[pc].T @ X_batch ----
            pi_sb = pi_pool.tile([128, 2, 2, C * PL], f8)
            for h in range(2):
                nc.sync.dma_start(pi_sb[:, h, :, :], pi_d[:, h, :, :])
            with tc.tile_pool(name="psA", bufs=2, space="PSUM") as psA_pool:
                bat = _batches(slot_cls)
                cur_tile, xt, acc = -1, None, None
                prev_cell = -1
                for (s0, w, c) in bat:
                    ti = s0 // XG
                    if ti != cur_tile:
                        xt = xin_pool.tile([128, XG, 2, 2, D], f8, tag="xt")
                        nc.sync.dma_start(
                            xt[:],
                            xs[ti * XG:(ti + 1) * XG]
                            .rearrange("s q h r d -> q s h r d"))
                        cur_tile = ti
                    cell = s0 // 4
                    if cell != prev_cell:
                        acc = psA_pool.tile([128, 2, D, 4], f32)  # 2 banks
                        prev_cell = cell
                    st = s0 % XG
                    o0 = s0 % 4
                    for pc in range(2):
                        for h in range(2):
                            nc.tensor.matmul(
                                acc[:, pc, :, o0:o0 + w],
                                pi_sb[:, h, :,
                                      c * PL + pc * 128:c * PL + (pc + 1) * 128],
                                xt[:, st:st + w, h, :, :]
                                .rearrange("q i r d -> q r d i"),
                                start=(h == 0), stop=(h == 1),
                                perf_mode=DR,
                            )
                    if s0 + w == 4 * cell + 4:
                        # Corner-turn the full cell: psum[pc, d, 4i] ->
                        # xp[q, d, pc, 4i] fp8; split pc across DVE and ACT.
                        g0 = 4 * cell
                        nc.vector.tensor_copy(xp[:, :, 0, g0:g0 + 4],
                                              acc[:, 0, :, :])
                        nc.scalar.copy(xp[:, :, 1, g0:g0 + 4],
                                       acc[:, 1, :, :])

            # ---- Stage B: C3[i, j] partial, contract (q_p, r_p, d) ----
            with tc.tile_pool(name="psB", bufs=1, space="PSUM") as psB_pool:
                accs = [[psB_pool.tile([128, 512], f32, name=f"accB_{ic}_{jh}")
                         for jh in range(2)]
                        for ic in range(2)]   # [i-chunk][j-half]
                for t in range(D // DG):
                    ytile = y_pool.tile([128, 2, DG, NY], f8)
                    nc.sync.dma_start(ytile[:], yt[:, :, t * DG:(t + 1) * DG, :])
                    for di in range(DG):
                        d = t * DG + di
                        st, sp = (d == 0), (d == D - 1)
                        for ic in range(2):
                            lhsT = xp[:, d, :, 128 * ic:128 * ic + 128]
                            for jh in range(2):
                                nc.tensor.matmul(
                                    accs[ic][jh][:],
                                    lhsT,
                                    ytile[:, :, di, 512 * jh:512 * jh + 512],
                                    start=st, stop=sp, perf_mode=DR)

            out_sb = out_pool.tile([128, 2, NY], f32)
            nc.vector.tensor_copy(out_sb[:, 0, 0:512], accs[0][0][:])
            nc.scalar.copy(out_sb[:, 0, 512:1024], accs[0][1][:])
            nc.vector.tensor_copy(out_sb[:, 1, 0:512], accs[1][0][:])
            nc.scalar.copy(out_sb[:, 1, 512:1024], accs[1][1][:])
            nc.sync.dma_start(c3.rearrange("(ic q) j -> q ic j", q=128), out_sb[:])

    nc.compile()
    _PROGRAM_CACHE[slot_cls] = nc
    return nc


def kernel(X, Y, pi, classe):
    global LAST_RUN
    assert X.shape == (NX, T, D) and Y.shape == (NY, TP, D)
    assert pi.shape == (C, T, TP) and classe.shape == (NX,)
    X = np.asarray(X, dtype=np.float32)
    Y = np.asarray(Y, dtype=np.float32)
    pi = np.asarray(pi, dtype=np.float32)
    classe = np.asarray(classe)

    slot_cls, perm, dummy_rows = _schedule(classe)
    nc = _build_program(slot_cls)

    # Host-side sharding + layout prep (all-contiguous device DMAs).
    pi8 = pi.astype(F8)
    Ypd = np.ascontiguousarray(Y.astype(F8).transpose(1, 2, 0))  # [p, d, j]
    pi_maps, yt_maps = [], []
    for h in range(2):
        # pi_p[q_t, h_t, r_t, cls*PL + p] for this p-half
        pi_p = np.ascontiguousarray(
            pi8[:, :, h * PL:(h + 1) * PL]
            .reshape(C, 2, 2, 128, PL).transpose(3, 1, 2, 0, 4)
        ).reshape(128, 2, 2, C * PL)
        # ytp[q_p, r_p, d, j]
        ytp = np.ascontiguousarray(
            Ypd[h * PL:(h + 1) * PL]
            .reshape(2, 128, D, NY).transpose(1, 0, 2, 3))
        pi_maps.append(pi_p)
        yt_maps.append(ytp)
    in_maps = []
    for g in range(GX):
        rows = perm[:, g]
        # xs[s, q_t, h_t, r_t, d]
        xk = np.ascontiguousarray(
            X[rows].astype(F8).reshape(R, 2, 2, 128, D)
            .transpose(0, 3, 1, 2, 4))
        for h in range(2):
            in_maps.append({"xs": xk, "pi_d": pi_maps[h], "yt": yt_maps[h]})

    trace = bool(os.environ.get("BASS_TRACE"))
    LAST_RUN = run_bass_kernel_spmd(nc, in_maps, list(range(N_CORES)),
                                    trace=trace)
    C3 = np.empty((NX, NY), np.float32)
    for g in range(GX):
        part = LAST_RUN.results[2 * g]["c3"] + LAST_RUN.results[2 * g + 1]["c3"]
        C3[perm[:, g]] = part
    if len(dummy_rows):
        # Exact f32 recompute of the mixed-class remainder rows.
        XPm = np.einsum("rtd,rtp->rpd", X[dummy_rows], pi[classe[dummy_rows]])
        C3[dummy_rows] = XPm.reshape(len(dummy_rows), -1) @ Y.reshape(NY, -1).T

    # Host epilogue: rank-1 corrections (0.15% of FLOPs).
    row_c = pi.sum(-1)                                 # [C, T]
    col_c = pi.sum(1)                                  # [C, TP]
    SX = np.einsum("itd,itd->it", X, X)                # [NX, T]
    SY = np.einsum("jpd,jpd->jp", Y, Y)                # [NY, TP]
    C1 = np.einsum("it,it->i", SX, row_c[classe])      # [NX]
    C2 = col_c @ SY.T                                  # [C, NY]
    return (C1[:, None] + C2[classe] - 2.0 * C3).astype(np.float32)


# revision 12
# speedup vs baseline: 1.6034x; 1.6034x over previous
"""Fused OT-DTW l2 cost-matrix kernel for Trainium2 (8 NeuronCores, SPMD).

mat_cost[i,j] = sum_{t,p,d} pi[cl(i)][t,p] * (X[i,t,d] - Y[j,p,d])^2
             = C1[i] + C2[cl(i), j] - 2 * C3[i,j]

with C3[i,j] = sum_{p,d} XP[i,p,d] * Y[j,p,d],  XP[i] = X[i].T @ pi[cl(i)].
The device computes the heavy parts (XP: ~69 GFLOP, C3: ~137 GFLOP) in
fp8e4m3 (pi is 0/1 so fp8 is exact for it; X/Y quantization error washes
out over the 65536-term contraction). The tiny rank-1 corrections C1/C2
(<0.2% of FLOPs) are applied on the host in fp32.

Sharding (4 row-groups x 2 p-halves): core k = 2g + h takes 256 rows of X
and contraction half p in [256h, 256h+256). Each core emits the partial
C3 over its p-half; the host adds the two partials per group. This halves
the Y stream per core (33.5MB, under the stage-B PE time) and splits
stage A across cores with zero duplication, putting per-core PE work at
the global fp8 roofline (~164us).

Everything runs fp8 DoubleRow (contraction 256/instr, 2 fp8/lane). The
dual-fp8 ISA forbids register-offset APs on the moving operand, so class
selection is baked statically: rows are grouped into 256 quadruples of 4
same-class rows (one row per group per slot -> all cores share one
slot->class schedule; programs are cached per schedule). Remainder rows
(<=24) land in dummy slots whose C3 rows the host recomputes exactly.

Stage A makes pi the stationary operand (static class offsets) and
streams X 4 rows at a time: out[p_chunk,(d,i)] = pi_chunk.T @ X_batch,
4 matmuls of free-dim 1024 per quad (LDWEIGHTS fully hidden), then
corner-turn casts PSUM->SBUF xp[q_p, d, r_p, i] fp8 split DVE/ACT.
Stage B contracts with p on partitions: for each d, lhsT = xp[:, d, :,
ic], rhs = ytp tile [q_p, r_p, 4d, 1024j] streamed through a 6-buffer
ring; 512 DR matmuls of free-dim 1024 accumulate into 4 PSUM banks.
A scratch-matmul burst at t=0 warms the PE clock-gate.
"""

import os
import sys
import types

import numpy as np
import ml_dtypes

NX, NY, T, TP, D, C = 1024, 1024, 512, 512, 128, 8
N_CORES = 8
GX = 4                      # row groups
R = NX // GX                # 256 rows (slots) per core
PL = TP // 2                # 256 local p (contraction half)
TC = T // 128               # 4 t-chunks
DG = 4                      # d-slices per Y DMA tile
XG = 8                      # slots per xs DMA tile
BF16 = ml_dtypes.bfloat16
F8 = ml_dtypes.float8_e4m3fn


def _ensure_axon_hooks():
    """concourse.bass_utils imports antenv.axon_hooks when tracing under
    axon; some images lack that submodule. Provide it, and register the
    NTFF profile hook if the boot path didn't."""
    try:
        import antenv
    except ImportError:
        return
    try:
        from antenv import axon_hooks  # noqa: F401
    except ImportError:
        mod = types.ModuleType("antenv.axon_hooks")
        mod._hook = None

        def _set(h):
            mod._hook = h

        def _get():
            return mod._hook

        mod.set_axon_ntff_profile_hook = _set
        mod.get_axon_ntff_profile_hook = _get
        sys.modules["antenv.axon_hooks"] = mod
        antenv.axon_hooks = mod
    from antenv.axon_hooks import (
        get_axon_ntff_profile_hook,
        set_axon_ntff_profile_hook,
    )

    if get_axon_ntff_profile_hook() is None:
        try:
            from trn_agent_boot.trn_boot import _ntff_profile_via_ctypes

            hook = _ntff_profile_via_ctypes("/opt/axon/libaxon_pjrt.so")
            if hook is not None:
                set_axon_ntff_profile_hook(hook)
        except Exception:
            pass


_ensure_axon_hooks()

import concourse.bass as bass  # noqa: E402  (bass.ds unused but kept for parity)
import concourse.tile as tile  # noqa: E402
from concourse import bacc, mybir  # noqa: E402
from concourse.bass_utils import run_bass_kernel_spmd  # noqa: E402

_PROGRAM_CACHE = {}
LAST_RUN = None  # BassKernelResults of the most recent kernel() call


def _schedule(classe):
    """Group rows into 256 quadruples (one row per group per slot).

    Returns (slot_cls, perm, dummy_rows): slot_cls[s] = baked class of
    slot s (remainder slots labeled 0), perm[s, g] = original row id at
    slot s of group g, dummy_rows = rows whose device result is replaced
    by an exact host recompute (mixed-class remainder quadruples).
    """
    slot_cls, quads, leftovers = [], [], []
    for c in range(C):
        rows = np.flatnonzero(classe == c)
        n4 = len(rows) // 4 * 4
        for k in range(0, n4, 4):
            quads.append(rows[k:k + 4])
            slot_cls.append(c)
        leftovers.extend(rows[n4:])
    leftovers = np.asarray(leftovers, dtype=np.int64)
    assert len(leftovers) % 4 == 0
    for k in range(0, len(leftovers), 4):
        quads.append(leftovers[k:k + 4])
        slot_cls.append(0)
    perm = np.stack(quads)                      # [256, 4]
    assert perm.shape == (R, GX)
    return tuple(slot_cls), perm, leftovers


def _batches(slot_cls):
    """Static-class batch pieces on the 4-slot grid: (s0, w, cls)."""
    out = []
    for b in range(R // 4):
        s = 4 * b
        while s < 4 * b + 4:
            c = slot_cls[s]
            w = 1
            while s + w < 4 * b + 4 and slot_cls[s + w] == c:
                w += 1
            out.append((s, w, int(c)))
            s += w
    return out


def _build_program(slot_cls):
    if slot_cls in _PROGRAM_CACHE:
        return _PROGRAM_CACHE[slot_cls]
    f8 = mybir.dt.float8e4
    f32 = mybir.dt.float32
    DR = mybir.MatmulPerfMode.DoubleRow
    nc = bacc.Bacc("TRN2", target_bir_lowering=False, debug=False,
                   num_devices=N_CORES)
    xs = nc.dram_tensor("xs", [R // 4, 128, 2, 2, 4, D], f8,
                        kind="ExternalInput").ap()
    pi_d = nc.dram_tensor("pi_d", [128, 2, 2, C * PL], f8,
                          kind="ExternalInput").ap()
    yt = nc.dram_tensor("yt", [128, 2, D, NY], f8, kind="ExternalInput").ap()
    c3 = nc.dram_tensor("c3", [R, NY], f32, kind="ExternalOutput").ap()

    with tile.TileContext(nc) as tc:
        with (
            tc.tile_pool(name="xpp", bufs=1) as xp_pool,
            tc.tile_pool(name="xin", bufs=8) as xin_pool,
            tc.tile_pool(name="pisb", bufs=1) as pi_pool,
            tc.tile_pool(name="yin", bufs=6) as y_pool,
            tc.tile_pool(name="outsb", bufs=1) as out_pool,
        ):
            # Resident transposed XP: xp[q_p, d, r_p, i] fp8 (64KB/part).
            xp = xp_pool.tile([128, D, 2, R], f8)

            # PE warmup: scratch matmuls at t=0 so the HAM clock-gate hits
            # 8/8 before the real matmuls start (values never read).
            with (
                tc.tile_pool(name="warm", bufs=1) as warm_pool,
                tc.tile_pool(name="warmps", bufs=1, space="PSUM") as warmps_pool,
            ):
                wsrc = warm_pool.tile([128, 512], f8)
                wacc = warmps_pool.tile([128, 512], f32)
                nc.gpsimd.memset(wsrc[:], 0.0)
                for w in range(14):
                    nc.tensor.matmul(wacc[:], wsrc[:, 0:128], wsrc[:],
                                     start=True, stop=True)

            # ---- Stage A: xp[:, :, pc, s] = pi_cl(s)[pc].T @ X_batch ----
            pi_sb = pi_pool.tile([128, 2, 2, C * PL], f8)
            for h in range(2):
                nc.sync.dma_start(pi_sb[:, h, :, :], pi_d[:, h, :, :])
            with tc.tile_pool(name="psA", bufs=2, space="PSUM") as psA_pool:
                bat = _batches(slot_cls)
                cur_cell, xt, acc = -1, None, None
                for (s0, w, c) in bat:
                    cell = s0 // 4
                    if cell != cur_cell:
                        # Per-cell X tile [q, h, r, i, d]: d innermost so the
                        # moving operand streams contiguous 128B runs (half-
                        # rate otherwise: one SBUF word-read per column).
                        xt = xin_pool.tile([128, 2, 2, 4, D], f8, tag="xt")
                        nc.sync.dma_start(xt[:], xs[cell])
                        acc = psA_pool.tile([128, 2, 4, D], f32)  # 2 banks
                        cur_cell = cell
                    o0 = s0 % 4
                    for pc in range(2):
                        for h in range(2):
                            nc.tensor.matmul(
                                acc[:, pc, o0:o0 + w, :],
                                pi_sb[:, h, :,
                                      c * PL + pc * 128:c * PL + (pc + 1) * 128],
                                xt[:, h, :, o0:o0 + w, :],
                                start=(h == 0), stop=(h == 1),
                                perf_mode=DR,
                            )
                    if s0 + w == 4 * cell + 4:
                        # Corner-turn the full cell: psum[pc, 4i, d] ->
                        # xp[q, d, pc, 4i] fp8; split pc across DVE and ACT.
                        g0 = 4 * cell
                        nc.vector.tensor_copy(
                            xp[:, :, 0, g0:g0 + 4],
                            acc[:, 0, :, :].rearrange("q i d -> q d i"))
                        nc.scalar.copy(
                            xp[:, :, 1, g0:g0 + 4],
                            acc[:, 1, :, :].rearrange("q i d -> q d i"))

            # ---- Stage B: C3[i, j] partial, contract (q_p, r_p, d) ----
            with tc.tile_pool(name="psB", bufs=1, space="PSUM") as psB_pool:
                accs = [[psB_pool.tile([128, 512], f32, name=f"accB_{ic}_{jh}")
                         for jh in range(2)]
                        for ic in range(2)]   # [i-chunk][j-half]
                for t in range(D // DG):
                    ytile = y_pool.tile([128, 2, DG, NY], f8)
                    nc.sync.dma_start(ytile[:], yt[:, :, t * DG:(t + 1) * DG, :])
                    for di in range(DG):
                        d = t * DG + di
                        st, sp = (d == 0), (d == D - 1)
                        for ic in range(2):
                            lhsT = xp[:, d, :, 128 * ic:128 * ic + 128]
                            for jh in range(2):
                                nc.tensor.matmul(
                                    accs[ic][jh][:],
                                    lhsT,
                                    ytile[:, :, di, 512 * jh:512 * jh + 512],
                                    start=st, stop=sp, perf_mode=DR)

            out_sb = out_pool.tile([128, 2, NY], f32)
            nc.vector.tensor_copy(out_sb[:, 0, 0:512], accs[0][0][:])
            nc.scalar.copy(out_sb[:, 0, 512:1024], accs[0][1][:])
            nc.vector.tensor_copy(out_sb[:, 1, 0:512], accs[1][0][:])
            nc.scalar.copy(out_sb[:, 1, 512:1024], accs[1][1][:])
            nc.sync.dma_start(c3.rearrange("(ic q) j -> q ic j", q=128), out_sb[:])

    nc.compile()
    _PROGRAM_CACHE[slot_cls] = nc
    return nc


def kernel(X, Y, pi, classe):
    global LAST_RUN
    assert X.shape == (NX, T, D) and Y.shape == (NY, TP, D)
    assert pi.shape == (C, T, TP) and classe.shape == (NX,)
    X = np.asarray(X, dtype=np.float32)
    Y = np.asarray(Y, dtype=np.float32)
    pi = np.asarray(pi, dtype=np.float32)
    classe = np.asarray(classe)

    slot_cls, perm, dummy_rows = _schedule(classe)
    nc = _build_program(slot_cls)

    # Host-side sharding + layout prep (all-contiguous device DMAs).
    pi8 = pi.astype(F8)
    Ypd = np.ascontiguousarray(Y.astype(F8).transpose(1, 2, 0))  # [p, d, j]
    pi_maps, yt_maps = [], []
    for h in range(2):
        # pi_p[q_t, h_t, r_t, cls*PL + p] for this p-half
        pi_p = np.ascontiguousarray(
            pi8[:, :, h * PL:(h + 1) * PL]
            .reshape(C, 2, 2, 128, PL).transpose(3, 1, 2, 0, 4)
        ).reshape(128, 2, 2, C * PL)
        # ytp[q_p, r_p, d, j]
        ytp = np.ascontiguousarray(
            Ypd[h * PL:(h + 1) * PL]
            .reshape(2, 128, D, NY).transpose(1, 0, 2, 3))
        pi_maps.append(pi_p)
        yt_maps.append(ytp)
    in_maps = []
    for g in range(GX):
        rows = perm[:, g]
        # xs[cell, q_t, h_t, r_t, i, d] (d innermost for full-rate streaming)
        xk = np.ascontiguousarray(
            X[rows].astype(F8).reshape(R // 4, 4, 2, 2, 128, D)
            .transpose(0, 4, 2, 3, 1, 5))
        for h in range(2):
            in_maps.append({"xs": xk, "pi_d": pi_maps[h], "yt": yt_maps[h]})

    trace = bool(os.environ.get("BASS_TRACE"))
    LAST_RUN = run_bass_kernel_spmd(nc, in_maps, list(range(N_CORES)),
                                    trace=trace)
    C3 = np.empty((NX, NY), np.float32)
    for g in range(GX):
        part = LAST_RUN.results[2 * g]["c3"] + LAST_RUN.results[2 * g + 1]["c3"]
        C3[perm[:, g]] = part
    if len(dummy_rows):
        # Exact f32 recompute of the mixed-class remainder rows.
        XPm = np.einsum("rtd,rtp->rpd", X[dummy_rows], pi[classe[dummy_rows]])
        C3[dummy_rows] = XPm.reshape(len(dummy_rows), -1) @ Y.reshape(NY, -1).T

    # Host epilogue: rank-1 corrections (0.15% of FLOPs).
    row_c = pi.sum(-1)                                 # [C, T]
    col_c = pi.sum(1)                                  # [C, TP]
    SX = np.einsum("itd,itd->it", X, X)                # [NX, T]
    SY = np.einsum("jpd,jpd->jp", Y, Y)                # [NY, TP]
    C1 = np.einsum("it,it->i", SX, row_c[classe])      # [NX]
    C2 = col_c @ SY.T                                  # [C, NY]
    return (C1[:, None] + C2[classe] - 2.0 * C3).astype(np.float32)


# revision 13
# speedup vs baseline: 1.7827x; 1.1118x over previous
"""Fused OT-DTW l2 cost-matrix kernel for Trainium2 (8 NeuronCores, SPMD).

mat_cost[i,j] = sum_{t,p,d} pi[cl(i)][t,p] * (X[i,t,d] - Y[j,p,d])^2
             = C1[i] + C2[cl(i), j] - 2 * C3[i,j]

with C3[i,j] = sum_{p,d} XP[i,p,d] * Y[j,p,d],  XP[i] = X[i].T @ pi[cl(i)].
The device computes the heavy parts (XP: ~69 GFLOP, C3: ~137 GFLOP) in
fp8e4m3 (pi is 0/1 so fp8 is exact for it; X/Y quantization error washes
out over the 65536-term contraction). The tiny rank-1 corrections C1/C2
(<0.2% of FLOPs) are applied on the host in fp32.

Sharding (4 row-groups x 2 p-halves): core k = 2g + h takes 256 rows of X
and contraction half p in [256h, 256h+256). Each core emits the partial
C3 over its p-half; the host adds the two partials per group. This halves
the Y stream per core (33.5MB, under the stage-B PE time) and splits
stage A across cores with zero duplication, putting per-core PE work at
the global fp8 roofline (~164us).

Everything runs fp8 DoubleRow (contraction 256/instr, 2 fp8/lane). The
dual-fp8 ISA forbids register-offset APs on the moving operand, so class
selection is baked statically: rows are grouped into 256 quadruples of 4
same-class rows (one row per group per slot -> all cores share one
slot->class schedule; programs are cached per schedule). Remainder rows
(<=24) land in dummy slots whose C3 rows the host recomputes exactly.

Stage A makes pi the stationary operand (static class offsets) and
streams X 4 rows at a time: out[p_chunk,(d,i)] = pi_chunk.T @ X_batch,
4 matmuls of free-dim 1024 per quad (LDWEIGHTS fully hidden), then
corner-turn casts PSUM->SBUF xp[q_p, d, r_p, i] fp8 split DVE/ACT.
Stage B contracts with p on partitions: for each d, lhsT = xp[:, d, :,
ic], rhs = ytp tile [q_p, r_p, 4d, 1024j] streamed through a 6-buffer
ring; 512 DR matmuls of free-dim 1024 accumulate into 4 PSUM banks.
A scratch-matmul burst at t=0 warms the PE clock-gate.
"""

import os
import sys
import types

import numpy as np
import ml_dtypes

NX, NY, T, TP, D, C = 1024, 1024, 512, 512, 128, 8
N_CORES = 8
GX = 4                      # row groups
R = NX // GX                # 256 rows (slots) per core
PL = TP // 2                # 256 local p (contraction half)
TC = T // 128               # 4 t-chunks
DG = 4                      # d-slices per Y DMA tile
XG = 8                      # slots per xs DMA tile
BF16 = ml_dtypes.bfloat16
F8 = ml_dtypes.float8_e4m3fn


def _ensure_axon_hooks():
    """concourse.bass_utils imports antenv.axon_hooks when tracing under
    axon; some images lack that submodule. Provide it, and register the
    NTFF profile hook if the boot path didn't."""
    try:
        import antenv
    except ImportError:
        return
    try:
        from antenv import axon_hooks  # noqa: F401
    except ImportError:
        mod = types.ModuleType("antenv.axon_hooks")
        mod._hook = None

        def _set(h):
            mod._hook = h

        def _get():
            return mod._hook

        mod.set_axon_ntff_profile_hook = _set
        mod.get_axon_ntff_profile_hook = _get
        sys.modules["antenv.axon_hooks"] = mod
        antenv.axon_hooks = mod
    from antenv.axon_hooks import (
        get_axon_ntff_profile_hook,
        set_axon_ntff_profile_hook,
    )

    if get_axon_ntff_profile_hook() is None:
        try:
            from trn_agent_boot.trn_boot import _ntff_profile_via_ctypes

            hook = _ntff_profile_via_ctypes("/opt/axon/libaxon_pjrt.so")
            if hook is not None:
                set_axon_ntff_profile_hook(hook)
        except Exception:
            pass


_ensure_axon_hooks()

import concourse.bass as bass  # noqa: E402  (bass.ds unused but kept for parity)
import concourse.tile as tile  # noqa: E402
from concourse import bacc, mybir  # noqa: E402
from concourse.bass_utils import run_bass_kernel_spmd  # noqa: E402

_PROGRAM_CACHE = {}
LAST_RUN = None  # BassKernelResults of the most recent kernel() call


def _schedule(classe):
    """Group rows into 256 quadruples (one row per group per slot).

    Returns (slot_cls, perm, dummy_rows): slot_cls[s] = baked class of
    slot s (remainder slots labeled 0), perm[s, g] = original row id at
    slot s of group g, dummy_rows = rows whose device result is replaced
    by an exact host recompute (mixed-class remainder quadruples).
    """
    slot_cls, quads, leftovers = [], [], []
    for c in range(C):
        rows = np.flatnonzero(classe == c)
        n4 = len(rows) // 4 * 4
        for k in range(0, n4, 4):
            quads.append(rows[k:k + 4])
            slot_cls.append(c)
        leftovers.extend(rows[n4:])
    leftovers = np.asarray(leftovers, dtype=np.int64)
    assert len(leftovers) % 4 == 0
    for k in range(0, len(leftovers), 4):
        quads.append(leftovers[k:k + 4])
        slot_cls.append(0)
    perm = np.stack(quads)                      # [256, 4]
    assert perm.shape == (R, GX)
    return tuple(slot_cls), perm, leftovers


def _batches(slot_cls):
    """Static-class batch pieces on the 4-slot grid: (s0, w, cls)."""
    out = []
    for b in range(R // 4):
        s = 4 * b
        while s < 4 * b + 4:
            c = slot_cls[s]
            w = 1
            while s + w < 4 * b + 4 and slot_cls[s + w] == c:
                w += 1
            out.append((s, w, int(c)))
            s += w
    return out


def _build_program(slot_cls):
    if slot_cls in _PROGRAM_CACHE:
        return _PROGRAM_CACHE[slot_cls]
    f8 = mybir.dt.float8e4
    f32 = mybir.dt.float32
    DR = mybir.MatmulPerfMode.DoubleRow
    nc = bacc.Bacc("TRN2", target_bir_lowering=False, debug=False,
                   num_devices=N_CORES)
    xs = nc.dram_tensor("xs", [R // 4, 128, 2, 2, 4, D], f8,
                        kind="ExternalInput").ap()
    pi_d = nc.dram_tensor("pi_d", [128, 2, 2, C * PL], f8,
                          kind="ExternalInput").ap()
    yt = nc.dram_tensor("yt", [128, 2, D, NY], f8, kind="ExternalInput").ap()
    c3 = nc.dram_tensor("c3", [R, NY], f32, kind="ExternalOutput").ap()

    with tile.TileContext(nc) as tc:
        with (
            tc.tile_pool(name="xpp", bufs=1) as xp_pool,
            tc.tile_pool(name="xin", bufs=8) as xin_pool,
            tc.tile_pool(name="pisb", bufs=1) as pi_pool,
            tc.tile_pool(name="yin", bufs=6) as y_pool,
            tc.tile_pool(name="outsb", bufs=1) as out_pool,
        ):
            # Resident transposed XP: xp[q_p, d, r_p, i] fp8 (64KB/part).
            xp = xp_pool.tile([128, D, 2, R], f8)

            # PE warmup: scratch matmuls at t=0 so the HAM clock-gate hits
            # 8/8 before the real matmuls start (values never read).
            with (
                tc.tile_pool(name="warm", bufs=1) as warm_pool,
                tc.tile_pool(name="warmps", bufs=1, space="PSUM") as warmps_pool,
            ):
                wsrc = warm_pool.tile([128, 512], f8)
                wacc = warmps_pool.tile([128, 512], f32)
                nc.gpsimd.memset(wsrc[:], 0.0)
                for w in range(14):
                    nc.tensor.matmul(wacc[:], wsrc[:, 0:128], wsrc[:],
                                     start=True, stop=True)

            # ---- Stage A: xp[:, :, pc, s] = pi_cl(s)[pc].T @ X_batch ----
            pi_sb = pi_pool.tile([128, 2, 2, C * PL], f8)
            for h in range(2):
                nc.sync.dma_start(pi_sb[:, h, :, :], pi_d[:, h, :, :])
            # 4 bufs x 2 banks = all 8 PSUM banks during stage A: the cell
            # pipeline rides 3 corner-turns ahead without stalling.
            with tc.tile_pool(name="psA", bufs=4, space="PSUM") as psA_pool:
                bat = _batches(slot_cls)
                cur_cell, xt, acc = -1, None, None
                for (s0, w, c) in bat:
                    cell = s0 // 4
                    if cell != cur_cell:
                        # Per-cell X tile [q, h, r, i, d]: d innermost so the
                        # moving operand streams contiguous 128B runs (half-
                        # rate otherwise: one SBUF word-read per column).
                        xt = xin_pool.tile([128, 2, 2, 4, D], f8, tag="xt")
                        nc.sync.dma_start(xt[:], xs[cell])
                        acc = psA_pool.tile([128, 2, 4, D], f32)  # 2 banks
                        cur_cell = cell
                    o0 = s0 % 4
                    for pc in range(2):
                        for h in range(2):
                            nc.tensor.matmul(
                                acc[:, pc, o0:o0 + w, :],
                                pi_sb[:, h, :,
                                      c * PL + pc * 128:c * PL + (pc + 1) * 128],
                                xt[:, h, :, o0:o0 + w, :],
                                start=(h == 0), stop=(h == 1),
                                perf_mode=DR,
                            )
                    if s0 + w == 4 * cell + 4:
                        # Corner-turn the full cell: psum[pc, 4i, d] ->
                        # xp[q, d, pc, 4i] fp8; split pc across DVE and ACT.
                        g0 = 4 * cell
                        nc.vector.tensor_copy(
                            xp[:, :, 0, g0:g0 + 4],
                            acc[:, 0, :, :].rearrange("q i d -> q d i"))
                        nc.scalar.copy(
                            xp[:, :, 1, g0:g0 + 4],
                            acc[:, 1, :, :].rearrange("q i d -> q d i"))

            # ---- Stage B: C3[i, j] partial, contract (q_p, r_p, d) ----
            with tc.tile_pool(name="psB", bufs=1, space="PSUM") as psB_pool:
                accs = [[psB_pool.tile([128, 512], f32, name=f"accB_{ic}_{jh}")
                         for jh in range(2)]
                        for ic in range(2)]   # [i-chunk][j-half]
                for t in range(D // DG):
                    ytile = y_pool.tile([128, 2, DG, NY], f8)
                    nc.sync.dma_start(ytile[:], yt[:, :, t * DG:(t + 1) * DG, :])
                    for di in range(DG):
                        d = t * DG + di
                        st, sp = (d == 0), (d == D - 1)
                        for ic in range(2):
                            lhsT = xp[:, d, :, 128 * ic:128 * ic + 128]
                            for jh in range(2):
                                nc.tensor.matmul(
                                    accs[ic][jh][:],
                                    lhsT,
                                    ytile[:, :, di, 512 * jh:512 * jh + 512],
                                    start=st, stop=sp, perf_mode=DR)

            out_sb = out_pool.tile([128, 2, NY], f32)
            nc.vector.tensor_copy(out_sb[:, 0, 0:512], accs[0][0][:])
            nc.scalar.copy(out_sb[:, 0, 512:1024], accs[0][1][:])
            nc.vector.tensor_copy(out_sb[:, 1, 0:512], accs[1][0][:])
            nc.scalar.copy(out_sb[:, 1, 512:1024], accs[1][1][:])
            nc.sync.dma_start(c3.rearrange("(ic q) j -> q ic j", q=128), out_sb[:])

    nc.compile()
    _PROGRAM_CACHE[slot_cls] = nc
    return nc


def kernel(X, Y, pi, classe):
    global LAST_RUN
    assert X.shape == (NX, T, D) and Y.shape == (NY, TP, D)
    assert pi.shape == (C, T, TP) and classe.shape == (NX,)
    X = np.asarray(X, dtype=np.float32)
    Y = np.asarray(Y, dtype=np.float32)
    pi = np.asarray(pi, dtype=np.float32)
    classe = np.asarray(classe)

    slot_cls, perm, dummy_rows = _schedule(classe)
    nc = _build_program(slot_cls)

    # Host-side sharding + layout prep (all-contiguous device DMAs).
    pi8 = pi.astype(F8)
    Ypd = np.ascontiguousarray(Y.astype(F8).transpose(1, 2, 0))  # [p, d, j]
    pi_maps, yt_maps = [], []
    for h in range(2):
        # pi_p[q_t, h_t, r_t, cls*PL + p] for this p-half
        pi_p = np.ascontiguousarray(
            pi8[:, :, h * PL:(h + 1) * PL]
            .reshape(C, 2, 2, 128, PL).transpose(3, 1, 2, 0, 4)
        ).reshape(128, 2, 2, C * PL)
        # ytp[q_p, r_p, d, j]
        ytp = np.ascontiguousarray(
            Ypd[h * PL:(h + 1) * PL]
            .reshape(2, 128, D, NY).transpose(1, 0, 2, 3))
        pi_maps.append(pi_p)
        yt_maps.append(ytp)
    in_maps = []
    for g in range(GX):
        rows = perm[:, g]
        # xs[cell, q_t, h_t, r_t, i, d] (d innermost for full-rate streaming)
        xk = np.ascontiguousarray(
            X[rows].astype(F8).reshape(R // 4, 4, 2, 2, 128, D)
            .transpose(0, 4, 2, 3, 1, 5))
        for h in range(2):
            in_maps.append({"xs": xk, "pi_d": pi_maps[h], "yt": yt_maps[h]})

    trace = bool(os.environ.get("BASS_TRACE"))
    LAST_RUN = run_bass_kernel_spmd(nc, in_maps, list(range(N_CORES)),
                                    trace=trace)
    C3 = np.empty((NX, NY), np.float32)
    for g in range(GX):
        part = LAST_RUN.results[2 * g]["c3"] + LAST_RUN.results[2 * g + 1]["c3"]
        C3[perm[:, g]] = part
    if len(dummy_rows):
        # Exact f32 recompute of the mixed-class remainder rows.
        XPm = np.einsum("rtd,rtp->rpd", X[dummy_rows], pi[classe[dummy_rows]])
        C3[dummy_rows] = XPm.reshape(len(dummy_rows), -1) @ Y.reshape(NY, -1).T

    # Host epilogue: rank-1 corrections (0.15% of FLOPs).
    row_c = pi.sum(-1)                                 # [C, T]
    col_c = pi.sum(1)                                  # [C, TP]
    SX = np.einsum("itd,itd->it", X, X)                # [NX, T]
    SY = np.einsum("jpd,jpd->jp", Y, Y)                # [NY, TP]
    C1 = np.einsum("it,it->i", SX, row_c[classe])      # [NX]
    C2 = col_c @ SY.T                                  # [C, NY]
    return (C1[:, None] + C2[classe] - 2.0 * C3).astype(np.float32)
